# revision 9
# baseline (speedup 1.0000x reference)
"""GQA attention kernel for 8 Trainium2 NeuronCores.

Sharding: batch x head-group. Core c handles batch b = c // 4 and head
group g = c % 4 (8 q heads 8g..8g+7, kv heads 2g, 2g+1). Each core
computes a partial output  attn_out_g[b] @ w_out[rows of g]  and the
host sums the 4 partials per batch.

v2: fully fused slab pipeline. The program is emitted as a flat
sequence of per-512-token-slab stages:

    A(0); for s: B(s), C(s-1), A(s+1)

with per-slab SBUF tiles so the Tile ready-heap scheduler can overlap
the ACT-bound attention phase (exp) with projection / out-projection
matmuls on the PE. On-chip layout is fully transposed: x^T arrives via
XBAR DMA transpose, q^T/k^T come straight out of the QKV^T projection,
scores are computed per head as S^T = K @ Q^T (row-tiled concurrent
pairs on the PE), the softmax denominator comes from an appended
ones-column in V, and normalization reads the PV PSUM directly with a
cross-partition-base DVE multiply.
"""

import numpy as np
import ml_dtypes

B, T, D = 2, 2048, 2048
H, KVH, HD = 32, 8, 64
KVD = KVH * HD  # 512
NCORES = 8
SCALE = 1.0 / np.sqrt(HD)

_CACHE = {}


def _build():
    import concourse.bass as bass
    import concourse.mybir as mybir
    import concourse.tile as tile
    from concourse import bacc

    f32 = mybir.dt.float32
    bf16 = mybir.dt.bfloat16
    AF = mybir.ActivationFunctionType
    OP = mybir.AluOpType

    nc = bacc.Bacc("TRN2", target_bir_lowering=False, debug=False)

    xb = nc.dram_tensor("xb", [T, D], bf16, kind="ExternalInput")
    wqk = nc.dram_tensor("wqk", [D, 640], bf16, kind="ExternalInput")
    wv = nc.dram_tensor("wv", [D, 128], bf16, kind="ExternalInput")
    wo = nc.dram_tensor("wo", [512, D], bf16, kind="ExternalInput")
    sinT = nc.dram_tensor("sinT", [128, T], bf16, kind="ExternalInput")
    cosT = nc.dram_tensor("cosT", [128, T], bf16, kind="ExternalInput")
    perm = nc.dram_tensor("perm", [128, 128], bf16, kind="ExternalInput")
    ident128 = nc.dram_tensor("ident128", [128, 128], bf16, kind="ExternalInput")
    masks = nc.dram_tensor("masks", [4 * 128, 512], bf16, kind="ExternalInput")
    outp = nc.dram_tensor("outp", [T, D], f32, kind="ExternalOutput")

    DT = D // 128   # 16 d-tiles
    NSLAB = 4       # token slabs of 512
    SLAB = 512

    with tile.TileContext(nc) as tc:
        with (
            tc.tile_pool(name="const", bufs=1) as cpool,
            tc.tile_pool(name="resid", bufs=1) as rpool,
            tc.tile_pool(name="px", bufs=2) as px,
            tc.tile_pool(name="pa", bufs=1, space="PSUM") as pa,
            tc.tile_pool(name="pr", bufs=3) as pr,
            tc.tile_pool(name="psc", bufs=2, space="PSUM") as psc,
            tc.tile_pool(name="ppv", bufs=4, space="PSUM") as ppv,
            tc.tile_pool(name="ppr", bufs=12) as ppr,
            tc.tile_pool(name="pden", bufs=3) as pden,
            tc.tile_pool(name="po", bufs=1, space="PSUM") as po_pool,
            tc.tile_pool(name="pc", bufs=3) as pc,
        ):
            # ---- resident constants ----
            wqk_sb = [cpool.tile([128, 640], bf16, tag=f"wqk{i}", name=f"wqk{i}") for i in range(DT)]
            wv_sb = [cpool.tile([128, 128], bf16, tag=f"wv{i}", name=f"wv{i}") for i in range(DT)]
            wo_sb = [cpool.tile([128, D], bf16, tag=f"wo{i}", name=f"wo{i}") for i in range(4)]
            sin_sb = cpool.tile([128, T], bf16, tag="sin")
            cos_sb = cpool.tile([128, T], bf16, tag="cos")
            perm_sb = cpool.tile([128, 128], bf16, tag="perm")
            id128_sb = cpool.tile([128, 128], bf16, tag="id128")
            mask_sb = [cpool.tile([128, 512], bf16, tag=f"mask{r}", name=f"mask{r}") for r in range(4)]
            ones_sb = cpool.tile([1, 64], bf16, tag="ones")

            def load_residents():
                # issued on the ACT HWDGE ring so the SP ring is free for
                # the x DMA-transposes; ordered by first use.
                wqk3 = wqk.rearrange("(o p) e -> p o e", p=128)
                wv3 = wv.rearrange("(o p) e -> p o e", p=128)
                wo3 = wo.rearrange("(o p) e -> p o e", p=128)
                nc.scalar.dma_start(perm_sb[:], perm[:])
                nc.scalar.dma_start(sin_sb[:], sinT[:])
                nc.scalar.dma_start(cos_sb[:], cosT[:])
                for i in range(DT):
                    nc.scalar.dma_start(wqk_sb[i][:], wqk3[:, i])
                for i in range(DT):
                    nc.scalar.dma_start(wv_sb[i][:], wv3[:, i])
                nc.scalar.dma_start(id128_sb[:], ident128[:])
                m4 = masks.rearrange("(r p) q -> r p q", p=128)
                for r in range(4):
                    nc.scalar.dma_start(mask_sb[r][:], m4[r])
                for i in range(4):
                    nc.scalar.dma_start(wo_sb[i][:], wo3[:, i])
                nc.gpsimd.memset(ones_sb[:], 1.0)

            # ---- persistent activations (per-slab tiles) ----
            # qk[e][s]: e=0..3 q head pairs, e=4 k; partitions 0:64 head A,
            # 64:128 head B; free dim = 512 tokens of slab s.
            qk = [[rpool.tile([128, SLAB], bf16, tag=f"qk{e}_{s}", name=f"qk{e}_{s}")
                   for s in range(NSLAB)] for e in range(5)]
            vnat = [rpool.tile([128, 130], bf16, tag=f"vn{k}", name=f"vn{k}")
                    for k in range(16)]
            attnT = [[rpool.tile([128, SLAB], bf16, tag=f"at{j}_{s}", name=f"at{j}_{s}")
                      for s in range(NSLAB)] for j in range(4)]

            xTs = {}

            def prefetch(s):
                xTs[s] = [px.tile([128, SLAB], bf16, tag=f"xT{d}", name=f"xT{d}_{s}")
                          for d in range(DT)]
                for d in range(DT):
                    nc.sync.dma_start_transpose(
                        xTs[s][d][:], xb[s * SLAB:(s + 1) * SLAB, d * 128:(d + 1) * 128])

            def phaseA(s):
                xT = xTs[s]
                sl = slice(s * SLAB, (s + 1) * SLAB)
                for e in range(6):
                    acc = pa.tile([128, SLAB], f32, tag="acc")
                    wsrc = wqk_sb if e < 5 else wv_sb
                    ecol = e * 128 if e < 5 else 0
                    for d in range(DT):
                        nc.tensor.matmul(
                            acc[:], wsrc[d][:, ecol:ecol + 128], xT[d][:],
                            start=(d == 0), stop=(d == DT - 1))
                    raw = pr.tile([128, SLAB], bf16, tag="raw")
                    nc.vector.tensor_copy(raw[:], acc[:])
                    if e == 5:
                        # vT -> PE transpose -> v natural (+ ones cols)
                        for i in range(4):
                            kt = 4 * s + i
                            vtp = po_pool.tile([128, 128], bf16, tag="po")
                            nc.tensor.transpose(
                                vtp[:], raw[:, i * 128:(i + 1) * 128], id128_sb[:])
                            nc.gpsimd.memset(vnat[kt][:], 1.0)
                            nc.vector.tensor_copy(vnat[kt][:, 0:64], vtp[:, 0:64])
                            nc.vector.tensor_copy(vnat[kt][:, 65:129], vtp[:, 64:128])
                        continue
                    rot = pa.tile([128, SLAB], f32, tag="acc")
                    nc.tensor.matmul(rot[:], perm_sb[:], raw[:], start=True, stop=True)
                    m2 = pr.tile([128, SLAB], bf16, tag="m2")
                    nc.vector.tensor_tensor(m2[:], raw[:], cos_sb[:, sl], OP.mult)
                    m1 = pr.tile([128, SLAB], bf16, tag="m1")
                    nc.vector.tensor_tensor(m1[:], rot[:], sin_sb[:, sl], OP.mult)
                    nc.vector.tensor_tensor(qk[e][s][:], m1[:], m2[:], OP.add)

            # phase B is software-pipelined at 4-kt-chunk granularity:
            # per unit, scores+exp for chunk u are emitted together with the
            # PV matmuls for chunk u-1 (whose probs are already in SBUF), so
            # the PE stream is long dep-free bursts; normalization trails two
            # units behind its row's last PV so its den copies are done.
            norm_pend = []

            def tick_norms(force=False):
                for ent in norm_pend:
                    ent[0] += 1
                while norm_pend and (norm_pend[0][0] >= 1 or force):
                    _, den, pv, pvB, jj, ss = norm_pend.pop(0)
                    bcp = psc.tile([128, SLAB], f32, tag="sc")
                    nc.tensor.matmul(
                        bcp[0:64, :], ones_sb[0:1, :], den[0:1, 0:512],
                        start=True, stop=True)
                    nc.tensor.matmul(
                        bcp[64:128, :], ones_sb[0:1, :], den[0:1, 512:1024],
                        start=True, stop=True)
                    rec = pden.tile([128, SLAB], f32, tag="rec")
                    nc.vector.reciprocal(rec[:], bcp[:])
                    nc.vector.tensor_tensor(
                        attnT[jj][ss][0:64, :], pv[0:64, :], rec[0:64, :], OP.mult)
                    nc.vector.tensor_tensor(
                        attnT[jj][ss][64:128, :], pvB[0:64, :], rec[64:128, :],
                        OP.mult)

            def phaseB(s):
                nkt = 4 * s + 4

                def emit_pv(ent):
                    pv, pvB, jj, probs, c = ent
                    for i, pk in enumerate(probs):
                        kt = 4 * c + i
                        nc.tensor.matmul(
                            pv[0:65], vnat[kt][:, 0:65], pk[:, 0:512],
                            start=(kt == 0), stop=(kt == nkt - 1))
                        nc.tensor.matmul(
                            pvB[0:65], vnat[kt][:, 65:130], pk[:, 512:1024],
                            start=(kt == 0), stop=(kt == nkt - 1))
                    if 4 * c + 4 == nkt:
                        den = pden.tile([1, 1024], bf16, tag="den")
                        nc.vector.tensor_copy(den[0:1, 0:512], pv[64:65, :])
                        nc.vector.tensor_copy(den[0:1, 512:1024], pvB[64:65, :])
                        norm_pend.append([0, den, pv, pvB, jj, s])

                prev = None
                for j in range(4):
                    pv = ppv.tile([128, SLAB], f32, tag="pv")
                    pvB = ppv.tile([128, SLAB], f32, tag="pv")
                    for c in range(s + 1):
                        tick_norms()
                        probs = []
                        for i in range(4):
                            kt = 4 * c + i
                            ks, ko = kt // 4, (kt % 4) * 128
                            scA = psc.tile([128, SLAB], f32, tag="sc")
                            scB = psc.tile([128, SLAB], f32, tag="sc")
                            nc.tensor.matmul(
                                scA[:], qk[4][ks][0:64, ko:ko + 128],
                                qk[j][s][0:64, :], start=True, stop=True)
                            nc.tensor.matmul(
                                scB[:], qk[4][ks][64:128, ko:ko + 128],
                                qk[j][s][64:128, :], start=True, stop=True)
                            pk = ppr.tile([128, 1024], bf16, tag="probs")
                            nc.scalar.activation(
                                pk[:, 0:512], scA[:], AF.Exp, scale=float(SCALE))
                            nc.scalar.activation(
                                pk[:, 512:1024], scB[:], AF.Exp, scale=float(SCALE))
                            if c == s:
                                nc.vector.tensor_tensor(
                                    pk[:, 0:512], pk[:, 0:512], mask_sb[i][:],
                                    OP.mult)
                                nc.vector.tensor_tensor(
                                    pk[:, 512:1024], pk[:, 512:1024], mask_sb[i][:],
                                    OP.mult)
                            probs.append(pk)
                        if prev is not None:
                            emit_pv(prev)
                        prev = (pv, pvB, j, probs, c)
                emit_pv(prev)
                tick_norms(force=True)

            def phaseC(s):
                for i in range(4):
                    it = 4 * s + i
                    for ns in range(4):
                        po = po_pool.tile([128, SLAB], f32, tag="po")
                        for j in range(4):
                            nc.tensor.matmul(
                                po[:],
                                attnT[j][s][:, i * 128:(i + 1) * 128],
                                wo_sb[j][:, ns * 512:(ns + 1) * 512],
                                start=(j == 0), stop=(j == 3))
                        ot = pc.tile([128, SLAB], f32, tag="ot")
                        nc.vector.tensor_copy(ot[:], po[:])
                        nc.gpsimd.dma_start(
                            outp[it * 128:(it + 1) * 128, ns * 512:(ns + 1) * 512],
                            ot[:])

            prefetch(0)
            load_residents()
            prefetch(1)
            phaseA(0)
            for s in range(NSLAB):
                phaseB(s)
                if s + 2 < NSLAB:
                    prefetch(s + 2)
                if s + 1 < NSLAB:
                    phaseA(s + 1)
                if s >= 1:
                    phaseC(s - 1)
            phaseC(NSLAB - 1)

    nc.finalize()
    return nc


def _host_inputs(x, sin, cos, w_qkv, w_out):
    bf = ml_dtypes.bfloat16
    sinT_np = np.concatenate([sin.T, sin.T], axis=0).astype(bf)  # [128, T]
    cosT_np = np.concatenate([cos.T, cos.T], axis=0).astype(bf)

    perm_np = np.zeros((128, 128), np.float32)
    for blk in range(2):
        for p in range(64):
            k = blk * 64 + ((p + 32) % 64)
            perm_np[k, blk * 64 + p] = -1.0 if p < 32 else 1.0
    perm_np = perm_np.astype(bf)
    id128_np = np.eye(128, dtype=np.float32).astype(bf)

    mask_np = np.zeros((4, 128, 512), np.float32)
    cix = np.arange(512)[None, :]
    pix = np.arange(128)[:, None]
    for r in range(4):
        mask_np[r] = (cix >= 128 * r + pix).astype(np.float32)
    mask_np = mask_np.reshape(512, 512).astype(bf)

    in_maps = []
    for c in range(NCORES):
        b, g = divmod(c, 4)
        cols = []
        for j in range(4):
            h1, h2 = 8 * g + j, 8 * g + 4 + j
            cols.append(w_qkv[:, 64 * h1:64 * h1 + 64])
            cols.append(w_qkv[:, 64 * h2:64 * h2 + 64])
        cols.append(w_qkv[:, D + 128 * g: D + 128 * g + 128])  # k heads 2g,2g+1
        wqk_np = np.concatenate(cols, axis=1).astype(bf)
        wv_np = w_qkv[:, D + KVD + 128 * g: D + KVD + 128 * g + 128].astype(bf)
        rows = []
        for j in range(4):
            h1, h2 = 8 * g + j, 8 * g + 4 + j
            rows.append(w_out[64 * h1:64 * h1 + 64, :])
            rows.append(w_out[64 * h2:64 * h2 + 64, :])
        wo_np = np.concatenate(rows, axis=0).astype(bf)
        in_maps.append({
            "xb": x[b].astype(bf),
            "wqk": wqk_np,
            "wv": wv_np,
            "wo": wo_np,
            "sinT": sinT_np,
            "cosT": cosT_np,
            "perm": perm_np,
            "ident128": id128_np,
            "masks": mask_np,
        })
    return in_maps


def kernel(x, sin, cos, w_qkv, w_out, _trace=False):
    from concourse.bass_utils import run_bass_kernel_spmd

    if "nc" not in _CACHE:
        _CACHE["nc"] = _build()
    nc = _CACHE["nc"]

    in_maps = _host_inputs(
        np.asarray(x), np.asarray(sin), np.asarray(cos),
        np.asarray(w_qkv), np.asarray(w_out))
    res = run_bass_kernel_spmd(
        nc, in_maps, core_ids=list(range(NCORES)), trace=_trace)
    out = np.zeros((B, T, D), np.float32)
    for c in range(NCORES):
        b = c // 4
        out[b] += res.results[c]["outp"]
    if _trace:
        kernel.last_result = res
    return out


# revision 10
# speedup vs baseline: 1.5571x; 1.5571x over previous
"""GQA attention kernel for 8 Trainium2 NeuronCores.

Sharding: batch x head-group. Core c handles batch b = c // 4 and head
group g = c % 4 (8 q heads 8g..8g+7, kv heads 2g, 2g+1). Each core
computes a partial output  attn_out_g[b] @ w_out[rows of g]  and the
host sums the 4 partials per batch.

On-chip layout is fully transposed: x^T arrives via bf16 XBAR DMA
transpose, q^T/k^T come straight out of the QKV^T projection, scores
are computed as S^T = K @ Q^T (softmax over the partition dim, with the
denominator produced by an appended ones-column in V), and the PV
output^T feeds the out-projection as lhsT.
"""

import numpy as np
import ml_dtypes

B, T, D = 2, 2048, 2048
H, KVH, HD = 32, 8, 64
KVD = KVH * HD  # 512
NCORES = 8
NEG = -3.0e38
SCALE = 1.0 / np.sqrt(HD)

_CACHE = {}


def _build():
    import concourse.bass as bass
    import concourse.mybir as mybir
    import concourse.tile as tile
    from concourse import bacc

    f32 = mybir.dt.float32
    bf16 = mybir.dt.bfloat16
    AF = mybir.ActivationFunctionType
    OP = mybir.AluOpType

    nc = bacc.Bacc("TRN2", target_bir_lowering=False, debug=False)

    xb = nc.dram_tensor("xb", [T, D], bf16, kind="ExternalInput")
    wqk = nc.dram_tensor("wqk", [D, 640], bf16, kind="ExternalInput")
    wv = nc.dram_tensor("wv", [D, 128], bf16, kind="ExternalInput")
    wo = nc.dram_tensor("wo", [512, D], bf16, kind="ExternalInput")
    sinT = nc.dram_tensor("sinT", [128, T], bf16, kind="ExternalInput")
    cosT = nc.dram_tensor("cosT", [128, T], bf16, kind="ExternalInput")
    perm = nc.dram_tensor("perm", [128, 128], bf16, kind="ExternalInput")
    ident = nc.dram_tensor("ident", [64, 64], bf16, kind="ExternalInput")
    ident128 = nc.dram_tensor("ident128", [128, 128], bf16, kind="ExternalInput")
    masks = nc.dram_tensor("masks", [4 * 128, 1024], bf16, kind="ExternalInput")
    outp = nc.dram_tensor("outp", [T, D], f32, kind="ExternalOutput")

    DT = D // 128   # 16 d-tiles
    NSLAB = 4       # token slabs of 512
    SLAB = 512
    NKT = T // 128  # 16 k token tiles

    with tile.TileContext(nc) as tc:
        with (
            tc.tile_pool(name="const", bufs=1) as cpool,
            tc.tile_pool(name="resid", bufs=1) as rpool,
        ):
            # ---- resident constants ----
            wqk_sb = [cpool.tile([128, 640], bf16, tag=f"wqk{i}", name=f"wqk{i}") for i in range(DT)]
            wv_sb = [cpool.tile([128, 128], bf16, tag=f"wv{i}", name=f"wv{i}") for i in range(DT)]
            wo_sb = [cpool.tile([128, D], bf16, tag=f"wo{i}", name=f"wo{i}") for i in range(4)]
            sin_sb = cpool.tile([128, T], bf16, tag="sin")
            cos_sb = cpool.tile([128, T], bf16, tag="cos")
            perm_sb = cpool.tile([128, 128], bf16, tag="perm")
            ident_sb = cpool.tile([64, 64], bf16, tag="ident")
            id128_sb = cpool.tile([128, 128], bf16, tag="id128")
            mask_sb = [cpool.tile([128, 1024], bf16, tag=f"mask{r}", name=f"mask{r}") for r in range(4)]
            ones_sb = cpool.tile([65, 64], bf16, tag="ones")

            wqk3 = wqk.rearrange("(o p) e -> p o e", p=128)
            wv3 = wv.rearrange("(o p) e -> p o e", p=128)
            wo3 = wo.rearrange("(o p) e -> p o e", p=128)
            for i in range(DT):
                nc.sync.dma_start(wqk_sb[i][:], wqk3[:, i])
                nc.sync.dma_start(wv_sb[i][:], wv3[:, i])
            for i in range(4):
                nc.sync.dma_start(wo_sb[i][:], wo3[:, i])
            nc.sync.dma_start(sin_sb[:], sinT[:])
            nc.sync.dma_start(cos_sb[:], cosT[:])
            nc.sync.dma_start(perm_sb[:], perm[:])
            nc.sync.dma_start(ident_sb[:], ident[:])
            nc.sync.dma_start(id128_sb[:], ident128[:])
            m4 = masks.rearrange("(r p) q -> r p q", p=128)
            for r in range(4):
                nc.sync.dma_start(mask_sb[r][:], m4[r])
            nc.gpsimd.memset(ones_sb[:], 1.0)

            # ---- persistent activations ----
            # qT tiles j=0..3: partitions 0:64 head j, 64:128 head j+4
            qkT = [rpool.tile([128, T], bf16, tag=f"qkT{e}", name=f"qkT{e}") for e in range(5)]
            vnat = [rpool.tile([128, 130], bf16, tag=f"vn{k}", name=f"vn{k}") for k in range(NKT)]
            attnT = [rpool.tile([128, T], bf16, tag=f"attnT{j}", name=f"attnT{j}") for j in range(4)]

            # ================= Phase A: projections =================
            with (
                tc.tile_pool(name="pa", bufs=2) as pa,
                tc.tile_pool(name="parope", bufs=3) as pr,
                tc.tile_pool(name="ps_qkv", bufs=2, space="PSUM") as ps_qkv,
                tc.tile_pool(name="ps_rot", bufs=2, space="PSUM") as ps_rot,
                tc.tile_pool(name="ps_v", bufs=2, space="PSUM") as ps_v,
            ):
                for s in range(NSLAB):
                    xT = [pa.tile([128, SLAB], bf16, tag=f"xT{d}", name=f"xT{d}") for d in range(DT)]
                    for d in range(DT):
                        nc.sync.dma_start_transpose(
                            xT[d][:], xb[s * SLAB:(s + 1) * SLAB, d * 128:(d + 1) * 128]
                        )
                    # q/k/v projection (transposed out) + rope
                    for e in range(6):
                        acc = ps_qkv.tile([128, SLAB], f32, tag="qkv")
                        wsrc = wqk_sb if e < 5 else wv_sb
                        ecol = e * 128 if e < 5 else 0
                        for d in range(DT):
                            nc.tensor.matmul(
                                acc[:], wsrc[d][:, ecol:ecol + 128], xT[d][:],
                                start=(d == 0), stop=(d == DT - 1),
                            )
                        raw = pr.tile([128, SLAB], bf16, tag="raw")
                        nc.vector.tensor_copy(raw[:], acc[:])
                        if e == 5:
                            # vT -> PE transpose -> v natural (+ ones cols)
                            for i in range(4):
                                kt = 4 * s + i
                                vtp = ps_v.tile([128, 128], bf16, tag="v")
                                nc.tensor.transpose(
                                    vtp[:], raw[:, i * 128:(i + 1) * 128], id128_sb[:])
                                nc.gpsimd.memset(vnat[kt][:], 1.0)
                                nc.vector.tensor_copy(vnat[kt][:, 0:64], vtp[:, 0:64])
                                nc.vector.tensor_copy(vnat[kt][:, 65:129], vtp[:, 64:128])
                            continue
                        rot = ps_rot.tile([128, SLAB], f32, tag="rot")
                        nc.tensor.matmul(rot[:], perm_sb[:], raw[:], start=True, stop=True)
                        m2 = pr.tile([128, SLAB], bf16, tag="m2")
                        nc.vector.tensor_tensor(
                            m2[:], raw[:], cos_sb[:, s * SLAB:(s + 1) * SLAB], OP.mult)
                        m1 = pr.tile([128, SLAB], bf16, tag="m1")
                        nc.vector.tensor_tensor(
                            m1[:], rot[:], sin_sb[:, s * SLAB:(s + 1) * SLAB], OP.mult)
                        nc.vector.tensor_tensor(
                            qkT[e][:, s * SLAB:(s + 1) * SLAB], m1[:], m2[:], OP.add)

            # ================= Phase B: attention =================
            with (
                tc.tile_pool(name="pb", bufs=3) as pb,
                tc.tile_pool(name="pbn", bufs=3) as pbn,
                tc.tile_pool(name="ps_sc", bufs=2, space="PSUM") as ps_sc,
                tc.tile_pool(name="ps_pv", bufs=3, space="PSUM") as ps_pv,
                tc.tile_pool(name="ps_bc", bufs=1, space="PSUM") as ps_bc,
            ):
                def emit_norm(pend):
                    den, stgA, stgB, jj, qq = pend
                    qsl2 = slice(qq * 512, (qq + 1) * 512)
                    bc = ps_bc.tile([128, 512], f32, tag="bc")
                    nc.tensor.matmul(
                        bc[0:64], ones_sb[64:65, :], den[64:65, 0:512],
                        start=True, stop=True)
                    nc.tensor.matmul(
                        bc[64:128], ones_sb[64:65, :], den[64:65, 512:1024],
                        start=True, stop=True)
                    rec = pbn.tile([128, 512], f32, tag="rec")
                    nc.vector.reciprocal(rec[:], bc[:])
                    nc.vector.tensor_tensor(
                        attnT[jj][0:64, qsl2], stgA[:], rec[0:64], OP.mult)
                    mvB = ps_bc.tile([128, 512], f32, tag="bc")
                    nc.tensor.matmul(
                        mvB[64:128], ident_sb[:], stgB[:], start=True, stop=True)
                    nc.vector.tensor_tensor(
                        attnT[jj][64:128, qsl2], mvB[64:128], rec[64:128], OP.mult)

                pending = None
                for j in range(4):
                    for qs in range(4):
                        nkt = 4 * qs + 4
                        qsl = slice(qs * 512, (qs + 1) * 512)
                        pv = ps_pv.tile([128, 512], f32, tag="pv")
                        pvB = ps_pv.tile([128, 512], f32, tag="pv")
                        probs = {}
                        # software pipeline: scores/exp one kt ahead of PV
                        for kt in range(nkt + 1):
                            if kt == 2 and pending is not None:
                                emit_norm(pending)
                                pending = None
                            if kt < nkt:
                                sc = ps_sc.tile([128, 1024], f32, tag="sc")
                                for h, base in ((0, 0), (1, 64)):
                                    nc.tensor.matmul(
                                        sc[:, h * 512:(h + 1) * 512],
                                        qkT[4][base:base + 64, kt * 128:(kt + 1) * 128],
                                        qkT[j][base:base + 64, qsl],
                                        start=True, stop=True,
                                    )
                                p = pb.tile([128, 1024], bf16, tag="probs")
                                nc.scalar.activation(p[:], sc[:], AF.Exp, scale=float(SCALE))
                                if kt >= 4 * qs:
                                    nc.vector.tensor_tensor(
                                        p[:], p[:], mask_sb[kt - 4 * qs][:], OP.mult)
                                probs[kt] = p
                            if kt >= 1:
                                k0 = kt - 1
                                nc.tensor.matmul(
                                    pv[0:65], vnat[k0][:, 0:65], probs[k0][:, 0:512],
                                    start=(k0 == 0), stop=(k0 == nkt - 1),
                                )
                                nc.tensor.matmul(
                                    pvB[0:65], vnat[k0][:, 65:130], probs[k0][:, 512:1024],
                                    start=(k0 == 0), stop=(k0 == nkt - 1),
                                )
                        # eager DVE part: free the pv banks quickly
                        den = pbn.tile([65, 1024], bf16, tag="den")
                        nc.vector.tensor_copy(den[64:65, 0:512], pv[64:65, :])
                        nc.vector.tensor_copy(den[64:65, 512:1024], pvB[64:65, :])
                        stgA = pbn.tile([64, 512], bf16, tag="stgA")
                        nc.vector.tensor_copy(stgA[:], pv[0:64])
                        stgB = pbn.tile([64, 512], bf16, tag="stgB")
                        nc.vector.tensor_copy(stgB[:], pvB[0:64])
                        pending = (den, stgA, stgB, j, qs)
                emit_norm(pending)

            # ================= Phase C: out projection =================
            with (
                tc.tile_pool(name="pc", bufs=3) as pc,
                tc.tile_pool(name="ps_o", bufs=4, space="PSUM") as ps_o,
            ):
                for i in range(16):
                    for ns in range(4):
                        po = ps_o.tile([128, 512], f32, tag="o")
                        for j in range(4):
                            nc.tensor.matmul(
                                po[:],
                                attnT[j][:, i * 128:(i + 1) * 128],
                                wo_sb[j][:, ns * 512:(ns + 1) * 512],
                                start=(j == 0), stop=(j == 3),
                            )
                        ot = pc.tile([128, 512], f32, tag="ot")
                        nc.vector.tensor_copy(ot[:], po[:])
                        nc.sync.dma_start(
                            outp[i * 128:(i + 1) * 128, ns * 512:(ns + 1) * 512], ot[:])

    nc.finalize()
    return nc


def _host_inputs(x, sin, cos, w_qkv, w_out):
    bf = ml_dtypes.bfloat16
    sinT_np = np.concatenate([sin.T, sin.T], axis=0).astype(bf)  # [128, T]
    cosT_np = np.concatenate([cos.T, cos.T], axis=0).astype(bf)

    perm_np = np.zeros((128, 128), np.float32)
    for blk in range(2):
        for p in range(64):
            k = blk * 64 + ((p + 32) % 64)
            perm_np[k, blk * 64 + p] = -1.0 if p < 32 else 1.0
    perm_np = perm_np.astype(bf)
    ident_np = np.eye(64, dtype=np.float32).astype(bf)
    id128_np = np.eye(128, dtype=np.float32).astype(bf)

    mask_np = np.zeros((4, 128, 1024), np.float32)
    cix = np.arange(512)[None, :]
    pix = np.arange(128)[:, None]
    for r in range(4):
        m = (cix >= 128 * r + pix).astype(np.float32)
        mask_np[r, :, 0:512] = m
        mask_np[r, :, 512:1024] = m
    mask_np = mask_np.reshape(512, 1024).astype(bf)

    in_maps = []
    for c in range(NCORES):
        b, g = divmod(c, 4)
        cols = []
        for j in range(4):
            h1, h2 = 8 * g + j, 8 * g + 4 + j
            cols.append(w_qkv[:, 64 * h1:64 * h1 + 64])
            cols.append(w_qkv[:, 64 * h2:64 * h2 + 64])
        cols.append(w_qkv[:, D + 128 * g: D + 128 * g + 128])  # k heads 2g,2g+1
        wqk_np = np.concatenate(cols, axis=1).astype(bf)
        wv_np = w_qkv[:, D + KVD + 128 * g: D + KVD + 128 * g + 128].astype(bf)
        rows = []
        for j in range(4):
            h1, h2 = 8 * g + j, 8 * g + 4 + j
            rows.append(w_out[64 * h1:64 * h1 + 64, :])
            rows.append(w_out[64 * h2:64 * h2 + 64, :])
        wo_np = np.concatenate(rows, axis=0).astype(bf)
        in_maps.append({
            "xb": x[b].astype(bf),
            "wqk": wqk_np,
            "wv": wv_np,
            "wo": wo_np,
            "sinT": sinT_np,
            "cosT": cosT_np,
            "perm": perm_np,
            "ident": ident_np,
            "ident128": id128_np,
            "masks": mask_np,
        })
    return in_maps


def kernel(x, sin, cos, w_qkv, w_out, _trace=False):
    from concourse.bass_utils import run_bass_kernel_spmd

    if "nc" not in _CACHE:
        _CACHE["nc"] = _build()
    nc = _CACHE["nc"]

    in_maps = _host_inputs(
        np.asarray(x), np.asarray(sin), np.asarray(cos),
        np.asarray(w_qkv), np.asarray(w_out))
    res = run_bass_kernel_spmd(
        nc, in_maps, core_ids=list(range(NCORES)), trace=_trace)
    out = np.zeros((B, T, D), np.float32)
    for c in range(NCORES):
        b = c // 4
        out[b] += res.results[c]["outp"]
    if _trace:
        kernel.last_result = res
    return out

